# revision 13
# baseline (speedup 1.0000x reference)
"""4D Conv-MLP (conv3^4 -> ReLU -> conv3^4) on 8 Trainium2 NeuronCores.

Sharding: core = b*4 + j  (batch b in {0,1}, H-slab j in {0..3}, 8 output rows
each). Each core computes its output slab independently: conv1 is recomputed on
a 1-row h halo (10 h rows from 12 x rows), so no cross-core communication is
needed. One SPMD program for all cores; per-core boundary behavior is driven by
data (host-zeroed x halos + h halo-row masks).

On-chip algorithm (implicit GEMM over the 81 taps, fp16 operands, fp32 PSUM):
  - x lives channel-on-partition as zero-padded flat planes per t ([16 D][12 H]
    [34 W], +1 lead pad), in two SBUF tiles of two shifted copies each:
    tileA = (x, x+1) and tileB = (x+2, x+36), so most K=128 matmuls contract
    two taps at once.
  - conv1: per (t, d): N=340 matmuls; each valid (kt, ku) block = 4 K=128
    pairs + 1 K=64 single (optimal for a 3x3 (kv, kw) grid with shift deltas
    {1, 34}); all-zero T/D edge taps are skipped; ReLU+bias on the Scalar
    engine writes fp16 h (pads skipped, out-of-image halo rows masked).
  - conv2: N=512 runs over d-pairs (N=256 at D edges, pad taps skipped);
    taps alternate PE column groups via tile_position (0,0)/(0,64) so two
    M=64 matmuls run concurrently; halves summed + bias on Scalar/DVE.
  Known pitfall baked into the structure: two partial-row (K=64) matmuls
  must never be adjacent in the PE stream (device wedge), and tile_size
  transitions stall the LDWEIGHTS pipeline, so singles are batched at the
  end of each conv1 accumulation chain.
"""

import numpy as np

B, C_IN, C_HID, C_OUT = 2, 64, 128, 64
T, D, H, W = 4, 16, 32, 32
NCORES, NJ = 8, 4
SH = H // NJ          # 8 out rows per slab
XH = SH + 4           # 12 x rows per slab
HHH = SH + 2          # 10 h rows per slab
XROW = 34             # padded W
XDP = 12 * XROW       # 408
XP = 1 + 16 * XDP + 7   # x plane size (real D rows only) = 6536
HD, HW_ = 18, 34
HP = HD * HHH * HW_   # h plane = 6120
N1 = HHH * XROW       # conv1 run = 340
N2 = 512              # conv2 run (2 d-rows)

_cache = {}


def _t_taps(t):
    return [kt for kt in range(3) if 0 <= t + kt - 1 < T]


def _g27(kt, ku, kv):
    return (kt * 3 + ku) * 3 + kv


def _g81(kt, ku, kv, kw):
    return ((kt * 3 + ku) * 3 + kv) * 3 + kw


def _make_host_arrays(x, w1, b1, w2, b2):
    x = np.asarray(x, np.float32)
    Xs, MTs, MBs = [], [], []
    for core in range(NCORES):
        b, j = divmod(core, NJ)
        h0 = SH * j
        slab = np.zeros((C_IN, T, D, XH, W), np.float32)
        lo, hi = h0 - 2, h0 + 10
        slo, shi = max(lo, 0), min(hi, H)
        slab[:, :, :, slo - lo:shi - lo, :] = x[b, :, :, :, slo:shi, :]
        plane = np.zeros((C_IN, T, D, XH, XROW), np.float32)
        plane[:, :, :, :, 1:33] = slab
        flat = plane.reshape(C_IN, T, D * XDP)
        X = np.zeros((C_IN, T, XP), np.float16)
        X[:, :, 1:1 + D * XDP] = flat
        Xs.append(X)
        MTs.append(np.full((128, 1), 0.0 if j == 0 else 1.0, np.float32))
        MBs.append(np.full((128, 1), 0.0 if j == NJ - 1 else 1.0, np.float32))

    w1 = np.asarray(w1, np.float32)
    w2 = np.asarray(w2, np.float32)
    # conv1 weights per tap, duplicated in both partition halves so either
    # K=64 row-tile chain ((0,0) or (64,0)) can use any tap
    W1D = np.zeros((128, 81, 128), np.float16)
    for kt in range(3):
        for ku in range(3):
            for kv in range(3):
                for kw in range(3):
                    g = _g81(kt, ku, kv, kw)
                    W1D[:64, g, :] = w1[:, :, kt, ku, kv, kw].T
                    W1D[64:, g, :] = w1[:, :, kt, ku, kv, kw].T
    W2 = np.zeros((128, 81, 64), np.float16)
    for kt in range(3):
        for ku in range(3):
            for kv in range(3):
                for kw in range(3):
                    gi = _g81(kt, ku, kv, kw)
                    W2[:, gi, :] = w2[:, :, kt, ku, kv, kw].T
    return dict(X=Xs, MT=MTs, MB=MBs,
                W1D=W1D.reshape(128, 81 * 128),
                W2=W2.reshape(128, 81 * 64),
                B1=np.asarray(b1, np.float32).reshape(128, 1),
                B2=np.asarray(b2, np.float32).reshape(64, 1))


def _build_module():
    import concourse.bass as bass
    import concourse.tile as tile
    from concourse import bacc, mybir

    fp16 = mybir.dt.float16
    fp32 = mybir.dt.float32

    nc = bacc.Bacc("TRN2", target_bir_lowering=False, debug=False, num_devices=1)
    x_d = nc.dram_tensor("x", [64, T, XP], fp16, kind="ExternalInput")
    w1d_d = nc.dram_tensor("w1d", [128, 81 * 128], fp16, kind="ExternalInput")
    w2_d = nc.dram_tensor("w2", [128, 81 * 64], fp16, kind="ExternalInput")
    b1_d = nc.dram_tensor("b1", [128, 1], fp32, kind="ExternalInput")
    b2_d = nc.dram_tensor("b2", [64, 1], fp32, kind="ExternalInput")
    mt_d = nc.dram_tensor("mt", [128, 1], fp32, kind="ExternalInput")
    mb_d = nc.dram_tensor("mb", [128, 1], fp32, kind="ExternalInput")
    y_d = nc.dram_tensor("y", [64, T, D * SH * W], fp32, kind="ExternalOutput")

    with tile.TileContext(nc) as tc:
        with (
            tc.tile_pool(name="xw", bufs=1) as xw,
            tc.tile_pool(name="hp", bufs=1) as hpool,
            tc.tile_pool(name="st", bufs=4) as stp,
            tc.tile_pool(name="p1", bufs=2, space="PSUM") as p1,
            tc.tile_pool(name="p2", bufs=4, space="PSUM") as p2,
        ):
            w1d = xw.tile([128, 81, 128], fp16)
            nc.sync.dma_start(w1d[:, :, :], w1d_d.ap())
            b1 = xw.tile([128, 1], fp32)
            nc.sync.dma_start(b1[:, :], b1_d.ap())

            # xa = (x, x+1): identical data in both partition halves, the
            # bottom half pre-shifted by one column so row-tile chain (64,0)
            # can address any tap at q-1; quarter-chunked, first chunks
            # DMAd first so conv1 can start early
            xa = xw.tile([128, T, XP], fp16)
            qs = [0, XP // 4, XP // 2, 3 * XP // 4, XP]

            def xchunk(t, ci):
                lo, hi = qs[ci], qs[ci + 1]
                for tdst, p0, s in ((xa, 0, 0), (xa, 64, 1)):
                    he = min(hi, XP - s)
                    nc.sync.dma_start(tdst[p0:p0 + 64, t, lo:he],
                                      x_d.ap()[:, t, lo + s:he + s])

            xchunk(0, 0)
            xchunk(1, 0)
            for t, ci in ((2, 0), (3, 0), (0, 1), (1, 1), (0, 2), (1, 2),
                          (0, 3), (1, 3), (2, 1), (3, 1), (2, 2), (3, 2),
                          (2, 3), (3, 3)):
                xchunk(t, ci)

            w2 = xw.tile([128, 81, 64], fp16)
            nc.sync.dma_start(w2[:, :, :], w2_d.ap())
            b2 = xw.tile([64, 1], fp32)
            nc.sync.dma_start(b2[:, :], b2_d.ap())
            mt = xw.tile([128, 1], fp32)
            nc.sync.dma_start(mt[:, :], mt_d.ap())
            mb = xw.tile([128, 1], fp32)
            nc.sync.dma_start(mb[:, :], mb_d.ap())

            ht = hpool.tile([128, T, HD, HHH, HW_], fp16)
            for t in range(T):
                nc.vector.memset(ht[:, t, :, :, :], 0.0)

            # ---- conv1 ----
            # every tap is a K=64, M=128 matmul; taps alternate between the
            # two PE row-tile chains (0,0) / (64,0), which stream
            # concurrently into separate PSUM banks. Chain B reads the
            # pre-shifted bottom half of xa, so its column offset is q-1
            # (tap s=0 is always placed on chain A). Halves are summed on
            # DVE, then bias+ReLU on Scalar writes fp16 h.
            for t in range(T):
                for d in range(D):
                    taps = [(kt, ku, kv, kw) for kt in _t_taps(t)
                            for ku in range(3) if 0 <= d + ku - 1 < D
                            for kv in range(3) for kw in range(3)]
                    psA = p1.tile([128, HHH, XROW], fp32)
                    psB = p1.tile([128, HHH, XROW], fp32)
                    # chain B reads the pre-shifted bottom half at q-1, so
                    # q == 0 taps must go on chain A; otherwise balance
                    ca, cb = [], []
                    for kt, ku, kv, kw in taps:
                        q = (d + ku - 1) * XDP + kv * XROW + kw
                        g = _g81(kt, ku, kv, kw)
                        tp = t + kt - 1
                        if q == 0 or len(ca) <= len(cb):
                            ca.append((tp, q, g))
                        else:
                            cb.append((tp, q, g))
                    for i in range(len(ca)):
                        tp, q, g = ca[i]
                        nc.tensor.matmul(
                            psA[:, :, :], w1d[0:64, g, :],
                            xa[0:64, tp, q:q + N1],
                            start=(i == 0), stop=(i == len(ca) - 1),
                            tile_position=(0, 0))
                        if i < len(cb):
                            tp, q, g = cb[i]
                            nc.tensor.matmul(
                                psB[:, :, :], w1d[64:128, g, :],
                                xa[64:128, tp, q - 1:q - 1 + N1],
                                start=(i == 0), stop=(i == len(cb) - 1),
                                tile_position=(64, 0))
                    stt = stp.tile([128, HHH, 32], fp32)
                    nc.scalar.activation(
                        stt[:, :, :], psB[:, :, 1:33],
                        mybir.ActivationFunctionType.Identity, bias=b1[:, 0:1])
                    nc.vector.tensor_add(stt[:, :, :], stt[:, :, :],
                                         psA[:, :, 1:33])
                    nc.vector.tensor_scalar_max(
                        ht[:, t, d + 1, :, 1:33], stt[:, :, :], 0.0)
                # zero out-of-image h halo rows (mask is 0 only on edge cores)
                nc.vector.tensor_scalar_mul(
                    ht[:, t, :, 0, 1:33], ht[:, t, :, 0, 1:33], mt[:, 0:1])
                nc.vector.tensor_scalar_mul(
                    ht[:, t, :, HHH - 1, 1:33], ht[:, t, :, HHH - 1, 1:33],
                    mb[:, 0:1])

            # ---- conv2 ----
            # runs: edge d=0 and d=15 alone (N=256, zero-pad taps skipped),
            # interior d as 7 pairs (N=512). Taps alternate between PE column
            # groups (psum partitions 0:64 / 64:128) so adjacent matmuls run
            # concurrently; halves summed via Scalar+DVE into the stage tile.
            runs = [(0, 1)] + [(d0, 2) for d0 in range(1, 15, 2)] + [(15, 1)]
            for t in range(T):
                for d0, nd in runs:
                    taps = [(kt, ku, kv, kw) for kt in _t_taps(t)
                            for ku in range(3) if 0 < d0 + ku < 17 or nd == 2
                            for kv in range(3) for kw in range(3)]
                    nn = nd * SH * W
                    lo = taps[0::2]
                    hi = taps[1::2]
                    ps = p2.tile([128, N2], fp32)
                    for i in range(len(lo)):
                        for half, base, tp_pos in ((lo, 0, (0, 0)),
                                                   (hi, 64, (0, 64))):
                            if i >= len(half):
                                continue
                            kt, ku, kv, kw = half[i]
                            gi = _g81(kt, ku, kv, kw)
                            rhs = ht[:, t + kt - 1, d0 + ku:d0 + ku + nd,
                                     kv:kv + SH, kw:kw + W]
                            nc.tensor.matmul(
                                ps[base:base + 64, 0:nn], w2[:, gi, :], rhs,
                                start=(i == 0), stop=(i == len(half) - 1),
                                tile_position=tp_pos)
                    st = stp.tile([64, N2], fp32)
                    nc.scalar.activation(
                        st[:, 0:nn], ps[64:128, 0:nn],
                        mybir.ActivationFunctionType.Identity, bias=b2[:, 0:1])
                    nc.vector.tensor_add(st[:, 0:nn], st[:, 0:nn],
                                         ps[0:64, 0:nn])
                    nc.sync.dma_start(
                        y_d.ap()[:, t, d0 * SH * W:d0 * SH * W + nn],
                        st[:, 0:nn])
    nc.compile()
    return nc


def _in_map(hostd, core):
    return {
        "x": hostd["X"][core], "mt": hostd["MT"][core],
        "mb": hostd["MB"][core],
        "w1d": hostd["W1D"], "w2": hostd["W2"],
        "b1": hostd["B1"], "b2": hostd["B2"],
    }


def kernel(x, w1, b1, w2, b2):
    from concourse.bass_utils import run_bass_kernel_spmd

    hostd = _make_host_arrays(x, w1, b1, w2, b2)
    if "nc" not in _cache:
        _cache["nc"] = _build_module()
    nc = _cache["nc"]

    in_maps = [_in_map(hostd, core) for core in range(NCORES)]
    res = run_bass_kernel_spmd(nc, in_maps, core_ids=list(range(NCORES)))

    y = np.zeros((B, C_OUT, T, D, H, W), np.float32)
    for core in range(NCORES):
        b, j = divmod(core, NJ)
        yc = res.results[core]["y"].reshape(C_OUT, T, D, SH, W)
        y[b, :, :, :, SH * j:SH * (j + 1), :] = yc
    return y



# revision 24
# speedup vs baseline: 1.3443x; 1.3443x over previous
"""4D Conv-MLP (conv3^4 -> ReLU -> conv3^4) on 8 Trainium2 NeuronCores.

Sharding: core = b*4 + j (batch b in {0,1}, H-slab j in {0..3}, 8 output rows
each). conv1 is recomputed on a 1-row h halo, so no cross-core communication.

F(2,3) Winograd along D for conv1 (input transform done on host, for free):
the ku tap dimension is replaced by 4 Winograd modes over 8 d-tiles of 2.
Each tap (kt,kv,kw) of each mode is a K=64, M=128 matmul; modes pair up on
the PE's two row-tile chains (tile_position (0,0)/(64,0)) which stream
concurrently, so C_in=64 runs at full array rate with no packed copies.
Mode pairs (0,1) then (2,3) accumulate per (t, dtile); modes 0,1 are staged
to SBUF (bias folded into mode 1), then the F(2,3) inverse + ReLU produce
the two h d-planes on DVE+Scalar while the PE streams the next tile.

conv2 stays direct: N=512 d-pair runs over 81 taps, K=128, M=64 (full rate);
taps alternate PE column groups via tile_position (0,0)/(0,64).
"""

import numpy as np

B, C_IN, C_HID, C_OUT = 2, 64, 128, 64
T, D, H, W = 4, 16, 32, 32
NCORES, NJ = 8, 4
SH = H // NJ          # 8 out rows per slab
XH = SH + 4           # 12 x rows per slab
HHH = SH + 2          # 10 h rows per slab
XROW = 34             # padded W
XDP = XH * XROW       # 408 = one (m, dtile) plane
MP = 1 + 2 * 8 * XDP + 7   # x-mode tile cols per half (2 modes)
HW_ = 34
N1 = HHH * XROW       # conv1 run = 340
N2 = 512              # conv2 run (2 dtiles x 8 x 32)

BT_W = np.array([[1, 0, -1, 0], [0, 1, 1, 0], [0, -1, 1, 0], [0, 1, 0, -1]],
                np.float32)
G_W = np.array([[1, 0, 0], [.5, .5, .5], [.5, -.5, .5], [0, 0, 1]], np.float32)

_cache = {}


def _t_taps(t):
    return [kt for kt in range(3) if 0 <= t + kt - 1 < T]


def _g27(kt, kv, kw):
    return (kt * 3 + kv) * 3 + kw


def _g81(kt, ku, kv, kw):
    return ((kt * 3 + ku) * 3 + kv) * 3 + kw


def _make_host_arrays(x, w1, b1, w2, b2):
    x = np.asarray(x, np.float32)
    Xs, MTs, MBs = [], [], []
    for core in range(NCORES):
        b, j = divmod(core, NJ)
        h0 = SH * j
        # x slab with h halo, then zero-pad d (+-1) and w (+-1)
        xpad = np.zeros((C_IN, T, D + 2, XH, XROW), np.float32)
        lo, hi = h0 - 2, h0 + 10
        slo, shi = max(lo, 0), min(hi, H)
        xpad[:, :, 1:17, slo - lo:shi - lo, 1:33] = x[b, :, :, :, slo:shi, :]
        # host F(2,3) forward transform along D: dtile jd covers out d
        # (2jd, 2jd+1), reads xpad d-rows 2jd..2jd+3
        xt = np.empty((C_IN, T, 4, 8, XH, XROW), np.float32)
        for jd in range(8):
            P = xpad[:, :, 2 * jd:2 * jd + 4]
            for m in range(4):
                xt[:, :, m, jd] = np.einsum('a,ctahw->cthw', BT_W[m], P)
        # chain A (PE rows 0:64) consumes modes 0,2; chain B modes 1,3
        XA = np.zeros((C_IN, T, MP), np.float16)
        XB = np.zeros((C_IN, T, MP), np.float16)
        XA[:, :, 1:1 + 2 * 8 * XDP] = xt[:, :, 0::2].reshape(C_IN, T, -1)
        XB[:, :, 1:1 + 2 * 8 * XDP] = xt[:, :, 1::2].reshape(C_IN, T, -1)
        Xs.append((XA, XB))
        MTs.append(np.full((128, 1), 0.0 if j == 0 else 1.0, np.float32))
        MBs.append(np.full((128, 1), 0.0 if j == NJ - 1 else 1.0, np.float32))

    w1 = np.asarray(w1, np.float32)
    w2 = np.asarray(w2, np.float32)
    # conv1 weights: F(2,3) transform along ku; top partition half serves
    # chain A (modes 0,2), bottom half chain B (modes 1,3)
    g1 = np.einsum('mu,oituvw->moitvw', G_W, w1)
    W1G = np.zeros((128, 2, 27, 128), np.float16)
    for pi in range(2):
        for kt in range(3):
            for kv in range(3):
                for kw in range(3):
                    g = _g27(kt, kv, kw)
                    W1G[:64, pi, g, :] = g1[2 * pi, :, :, kt, kv, kw].T
                    W1G[64:, pi, g, :] = g1[2 * pi + 1, :, :, kt, kv, kw].T
    # conv2 weights: F(2,3) transform along ku, per (mode, kt, kv, kw)
    g2 = np.einsum('mu,oituvw->moitvw', G_W, w2)
    W2G = np.zeros((128, 4, 27, 64), np.float16)
    for m in range(4):
        for kt in range(3):
            for kv in range(3):
                for kw in range(3):
                    g = _g27(kt, kv, kw)
                    W2G[:, m, g, :] = g2[m, :, :, kt, kv, kw].T
    return dict(X=Xs, MT=MTs, MB=MBs,
                W1G=W1G.reshape(128, 2 * 27 * 128),
                W2G=W2G.reshape(128, 4 * 27 * 64),
                B1=np.asarray(b1, np.float32).reshape(128, 1),
                B2=np.asarray(b2, np.float32).reshape(64, 1))


def _build_module():
    import concourse.bass as bass
    import concourse.tile as tile
    from concourse import bacc, mybir

    fp16 = mybir.dt.float16
    fp32 = mybir.dt.float32

    nc = bacc.Bacc("TRN2", target_bir_lowering=False, debug=False, num_devices=1)
    xA_d = nc.dram_tensor("xA", [64, T, MP], fp16, kind="ExternalInput")
    xB_d = nc.dram_tensor("xB", [64, T, MP], fp16, kind="ExternalInput")
    w1g_d = nc.dram_tensor("w1g", [128, 2 * 27 * 128], fp16, kind="ExternalInput")
    w2g_d = nc.dram_tensor("w2g", [128, 4 * 27 * 64], fp16, kind="ExternalInput")
    b1_d = nc.dram_tensor("b1", [128, 1], fp32, kind="ExternalInput")
    b2_d = nc.dram_tensor("b2", [64, 1], fp32, kind="ExternalInput")
    mt_d = nc.dram_tensor("mt", [128, 1], fp32, kind="ExternalInput")
    mb_d = nc.dram_tensor("mb", [128, 1], fp32, kind="ExternalInput")
    y_d = nc.dram_tensor("y", [64, T, D * SH * W], fp32, kind="ExternalOutput")

    with tile.TileContext(nc) as tc:
        with (
            tc.tile_pool(name="xw", bufs=1) as xw,
            tc.tile_pool(name="hp", bufs=1) as hpool,
            tc.tile_pool(name="st", bufs=4) as stp,
            tc.tile_pool(name="p1", bufs=2, space="PSUM") as p1,
            tc.tile_pool(name="p2", bufs=2, space="PSUM") as p2,
        ):
            w1g = xw.tile([128, 2, 27, 128], fp16)
            nc.sync.dma_start(w1g[:, :, :, :], w1g_d.ap())
            b1 = xw.tile([128, 1], fp32)
            nc.sync.dma_start(b1[:, :], b1_d.ap())

            # x-mode planes: top partition half holds modes (0,2) for chain
            # A, bottom half modes (1,3) for chain B. The h-mode planes for
            # conv2's pair-1 reuse this region (tag alias) once conv1 is done.
            xq = xw.tile([128, T, MP], fp16, tag="xh")
            for t in (0, 1):
                nc.sync.dma_start(xq[0:64, t, :], xA_d.ap()[:, t, :])
                nc.sync.dma_start(xq[64:128, t, :], xB_d.ap()[:, t, :])
            mt = xw.tile([128, 1], fp32)
            nc.sync.dma_start(mt[:, :], mt_d.ap())
            mb = xw.tile([128, 1], fp32)
            nc.sync.dma_start(mb[:, :], mb_d.ap())
            for t in (2, 3):
                nc.sync.dma_start(xq[0:64, t, :], xA_d.ap()[:, t, :])
                nc.sync.dma_start(xq[64:128, t, :], xB_d.ap()[:, t, :])

            w2g = xw.tile([128, 4, 27, 64], fp16)
            nc.sync.dma_start(w2g[:, :, :, :], w2g_d.ap())
            b2 = xw.tile([64, 1], fp32)
            nc.sync.dma_start(b2[:, :], b2_d.ap())

            ht = hpool.tile([128, T, D, HHH, HW_], fp16)
            for t in range(T):
                nc.vector.memset(ht[:, t, :, :, 0:1], 0.0)
                nc.vector.memset(ht[:, t, :, :, 33:34], 0.0)
            # h-mode planes for conv2 pair 0 (modes 0,1); pair 1 (modes 2,3)
            # lives in hb1, aliased over xq
            hb0 = hpool.tile([128, T, 2, 8, HHH, HW_], fp16)

            # ---- conv1 (D-Winograd) ----
            # per (t, dtile): mode pair (0,1) accumulates on chains A/B,
            # modes staged to SBUF (bias folded into mode 1), then pair
            # (2,3); inverse: h[2j] = M0+M1+M2, h[2j+1] = M1-M2-M3
            for t in range(T):
                taps = [(kt, kv, kw) for kt in _t_taps(t)
                        for kv in range(3) for kw in range(3)]
                n = len(taps)
                for j in range(8):
                    ms0 = stp.tile([128, HHH, 32], fp32)
                    ms1 = stp.tile([128, HHH, 32], fp32)
                    for pair in range(2):
                        psA = p1.tile([128, HHH, XROW], fp32)
                        psB = p1.tile([128, HHH, XROW], fp32)
                        for i, (kt, kv, kw) in enumerate(taps):
                            tp = t + kt - 1
                            g = _g27(kt, kv, kw)
                            q = (pair * 8 + j) * XDP + kv * XROW + kw
                            nc.tensor.matmul(
                                psA[:, :, :], w1g[0:64, pair, g, :],
                                xq[0:64, tp, q:q + N1],
                                start=(i == 0), stop=(i == n - 1),
                                tile_position=(0, 0))
                            nc.tensor.matmul(
                                psB[:, :, :], w1g[64:128, pair, g, :],
                                xq[64:128, tp, q:q + N1],
                                start=(i == 0), stop=(i == n - 1),
                                tile_position=(64, 0))
                        if pair == 0:
                            # stage modes 0,1; fold bias into mode 1 so both
                            # inverse outputs carry it exactly once
                            nc.vector.tensor_scalar_add(
                                ms0[:, :, :], psA[:, :, 1:33], 0.0)
                            nc.scalar.activation(
                                ms1[:, :, :], psB[:, :, 1:33],
                                mybir.ActivationFunctionType.Identity,
                                bias=b1[:, 0:1])
                        else:
                            nc.vector.tensor_add(ms0[:, :, :], ms0[:, :, :],
                                                 ms1[:, :, :])
                            nc.vector.tensor_add(ms0[:, :, :], ms0[:, :, :],
                                                 psA[:, :, 1:33])
                            nc.scalar.activation(
                                ht[:, t, 2 * j, :, 1:33], ms0[:, :, :],
                                mybir.ActivationFunctionType.Relu)
                            nc.vector.tensor_sub(ms1[:, :, :], ms1[:, :, :],
                                                 psA[:, :, 1:33])
                            nc.vector.tensor_sub(ms1[:, :, :], ms1[:, :, :],
                                                 psB[:, :, 1:33])
                            nc.scalar.activation(
                                ht[:, t, 2 * j + 1, :, 1:33], ms1[:, :, :],
                                mybir.ActivationFunctionType.Relu)
                # zero out-of-image h halo rows (mask is 0 only on edge cores)
                nc.vector.tensor_scalar_mul(
                    ht[:, t, :, 0, 1:33], ht[:, t, :, 0, 1:33], mt[:, 0:1])
                nc.vector.tensor_scalar_mul(
                    ht[:, t, :, HHH - 1, 1:33], ht[:, t, :, HHH - 1, 1:33],
                    mb[:, 0:1])
                # conv2 forward F(2,3) transform along d, pair-0 modes:
                # m0[jd] = h[2jd-1]-h[2jd+1] (h[-1]=0), m1[jd] = h[2jd]+h[2jd+1]
                nc.vector.tensor_scalar_mul(
                    hb0[:, t, 0, 0, :, :], ht[:, t, 1, :, :], -1.0)
                nc.vector.tensor_sub(
                    hb0[:, t, 0, 1:8, :, :], ht[:, t, 1:14:2, :, :],
                    ht[:, t, 3:16:2, :, :])
                nc.vector.tensor_add(
                    hb0[:, t, 1, :, :, :], ht[:, t, 0:15:2, :, :],
                    ht[:, t, 1:16:2, :, :])

            # pair-1 modes (reuse the x-mode region, conv1 is done with it):
            # m2[jd] = h[2jd+1]-h[2jd], m3[jd] = h[2jd]-h[2jd+2] (h[16]=0)
            hb1 = xw.tile([128, T, 2, 8, HHH, HW_], fp16, tag="xh")
            for t in range(T):
                nc.vector.tensor_sub(
                    hb1[:, t, 0, :, :, :], ht[:, t, 1:16:2, :, :],
                    ht[:, t, 0:15:2, :, :])
                nc.vector.tensor_sub(
                    hb1[:, t, 1, 0:7, :, :], ht[:, t, 0:13:2, :, :],
                    ht[:, t, 2:15:2, :, :])
                nc.vector.tensor_copy(
                    hb1[:, t, 1, 7, :, :], ht[:, t, 14, :, :])

            # ---- conv2 (D-Winograd) ----
            # per (t, jd-pair): mode pair (0,1) then (2,3); within a pair the
            # two modes run on PE column groups (0,0)/(0,64) with K=128,
            # N=512 (2 dtiles x 8 rows x 32). Inverse + bias on DVE/Scalar:
            # y[4jp+2js+0] = M0+M1+M2+b2, y[4jp+2js+1] = M1-M2-M3+b2
            for t in range(T):
                taps2 = [(kt, kv, kw) for kt in _t_taps(t)
                         for kv in range(3) for kw in range(3)]
                n2 = len(taps2)
                for jp in range(4):
                    ps1 = p2.tile([128, N2], fp32)
                    ps2t = p2.tile([128, N2], fp32)
                    for pst, hbp in ((ps1, hb0), (ps2t, hb1)):
                        mb_ = 0 if pst is ps1 else 2
                        for i, (kt, kv, kw) in enumerate(taps2):
                            tp = t + kt - 1
                            g = _g27(kt, kv, kw)
                            for mi, base, tpos in ((0, 0, (0, 0)),
                                                   (1, 64, (0, 64))):
                                rhs = hbp[:, tp, mi, 2 * jp:2 * jp + 2,
                                          kv:kv + SH, kw:kw + W]
                                nc.tensor.matmul(
                                    pst[base:base + 64, 0:N2],
                                    w2g[:, mb_ + mi, g, :], rhs,
                                    start=(i == 0), stop=(i == n2 - 1),
                                    tile_position=tpos)
                    s0 = stp.tile([64, N2], fp32, bufs=2)
                    s1 = stp.tile([64, N2], fp32, bufs=2)
                    nc.vector.tensor_scalar_add(s0[:, :], ps1[0:64, :], 0.0)
                    nc.scalar.activation(
                        s1[:, :], ps1[64:128, :],
                        mybir.ActivationFunctionType.Identity, bias=b2[:, 0:1])
                    nc.vector.tensor_add(s0[:, :], s0[:, :], s1[:, :])
                    nc.vector.tensor_add(s0[:, :], s0[:, :], ps2t[0:64, :])
                    nc.vector.tensor_sub(s1[:, :], s1[:, :], ps2t[0:64, :])
                    nc.vector.tensor_sub(s1[:, :], s1[:, :], ps2t[64:128, :])
                    qy = 4 * jp * 256
                    nc.sync.dma_start(y_d.ap()[:, t, qy:qy + 256],
                                      s0[:, 0:256])
                    nc.sync.dma_start(y_d.ap()[:, t, qy + 512:qy + 768],
                                      s0[:, 256:512])
                    nc.sync.dma_start(y_d.ap()[:, t, qy + 256:qy + 512],
                                      s1[:, 0:256])
                    nc.sync.dma_start(y_d.ap()[:, t, qy + 768:qy + 1024],
                                      s1[:, 256:512])
    nc.compile()
    return nc


def _in_map(hostd, core):
    return {
        "xA": hostd["X"][core][0], "xB": hostd["X"][core][1],
        "mt": hostd["MT"][core], "mb": hostd["MB"][core],
        "w1g": hostd["W1G"], "w2g": hostd["W2G"],
        "b1": hostd["B1"], "b2": hostd["B2"],
    }


def kernel(x, w1, b1, w2, b2):
    from concourse.bass_utils import run_bass_kernel_spmd

    hostd = _make_host_arrays(x, w1, b1, w2, b2)
    if "nc" not in _cache:
        _cache["nc"] = _build_module()
    nc = _cache["nc"]

    in_maps = [_in_map(hostd, core) for core in range(NCORES)]
    res = run_bass_kernel_spmd(nc, in_maps, core_ids=list(range(NCORES)))

    y = np.zeros((B, C_OUT, T, D, H, W), np.float32)
    for core in range(NCORES):
        b, j = divmod(core, NJ)
        yc = res.results[core]["y"].reshape(C_OUT, T, D, SH, W)
        y[b, :, :, :, SH * j:SH * (j + 1), :] = yc
    return y


# revision 28
# speedup vs baseline: 1.4267x; 1.0613x over previous
"""4D Conv-MLP (conv3^4 -> ReLU -> conv3^4) on 8 Trainium2 NeuronCores.

Sharding: core = b*4 + j (batch b in {0,1}, H-slab j in {0..3}, 8 output rows
each). conv1 is recomputed on a 1-row h halo, so no cross-core communication.

F(2,3) Winograd along D for conv1 (input transform done on host, for free):
the ku tap dimension is replaced by 4 Winograd modes over 8 d-tiles of 2.
Each tap (kt,kv,kw) of each mode is a K=64, M=128 matmul; modes pair up on
the PE's two row-tile chains (tile_position (0,0)/(64,0)) which stream
concurrently, so C_in=64 runs at full array rate with no packed copies.
Mode pairs (0,1) then (2,3) accumulate per (t, dtile); modes 0,1 are staged
to SBUF (bias folded into mode 1), then the F(2,3) inverse + ReLU produce
the two h d-planes on DVE+Scalar while the PE streams the next tile.

conv2 stays direct: N=512 d-pair runs over 81 taps, K=128, M=64 (full rate);
taps alternate PE column groups via tile_position (0,0)/(0,64).
"""

import numpy as np

B, C_IN, C_HID, C_OUT = 2, 64, 128, 64
T, D, H, W = 4, 16, 32, 32
NCORES, NJ = 8, 4
SH = H // NJ          # 8 out rows per slab
XH = SH + 4           # 12 x rows per slab
HHH = SH + 2          # 10 h rows per slab
XROW = 34             # padded W
XDP = XH * XROW       # 408 = one (m, dtile) plane
MP = 1 + 2 * 8 * XDP + 7   # x-mode tile cols per half (2 modes)
HW_ = 34
N1 = HHH * XROW       # conv1 run = 340
N2 = 512              # conv2 run (2 dtiles x 8 x 32)

BT_W = np.array([[1, 0, -1, 0], [0, 1, 1, 0], [0, -1, 1, 0], [0, 1, 0, -1]],
                np.float32)
G_W = np.array([[1, 0, 0], [.5, .5, .5], [.5, -.5, .5], [0, 0, 1]], np.float32)

_cache = {}


def _t_taps(t):
    return [kt for kt in range(3) if 0 <= t + kt - 1 < T]


def _g27(kt, kv, kw):
    return (kt * 3 + kv) * 3 + kw


def _g81(kt, ku, kv, kw):
    return ((kt * 3 + ku) * 3 + kv) * 3 + kw


def _make_host_arrays(x, w1, b1, w2, b2):
    x = np.asarray(x, np.float32)
    Xs, MTs, MBs = [], [], []
    for core in range(NCORES):
        b, j = divmod(core, NJ)
        h0 = SH * j
        # x slab with h halo, then zero-pad d (+-1) and w (+-1)
        xpad = np.zeros((C_IN, T, D + 2, XH, XROW), np.float32)
        lo, hi = h0 - 2, h0 + 10
        slo, shi = max(lo, 0), min(hi, H)
        xpad[:, :, 1:17, slo - lo:shi - lo, 1:33] = x[b, :, :, :, slo:shi, :]
        # host F(2,3) forward transform along D: dtile jd covers out d
        # (2jd, 2jd+1), reads xpad d-rows 2jd..2jd+3
        xt = np.empty((C_IN, T, 4, 8, XH, XROW), np.float32)
        for jd in range(8):
            P = xpad[:, :, 2 * jd:2 * jd + 4]
            for m in range(4):
                xt[:, :, m, jd] = np.einsum('a,ctahw->cthw', BT_W[m], P)
        # chain A (PE rows 0:64) consumes modes 0,2; chain B modes 1,3
        XA = xt[:, :, 0::2].reshape(C_IN, T, -1).astype(np.float16)
        XB = xt[:, :, 1::2].reshape(C_IN, T, -1).astype(np.float16)
        Xs.append((XA, XB))
        MTs.append(np.full((128, 1), 0.0 if j == 0 else 1.0, np.float32))
        MBs.append(np.full((128, 1), 0.0 if j == NJ - 1 else 1.0, np.float32))

    w1 = np.asarray(w1, np.float32)
    w2 = np.asarray(w2, np.float32)
    # conv1 weights: F(2,3) transform along ku; top partition half serves
    # chain A (modes 0,2), bottom half chain B (modes 1,3)
    g1 = np.einsum('mu,oituvw->moitvw', G_W, w1)
    W1G = np.zeros((128, 2, 27, 128), np.float16)
    for pi in range(2):
        for kt in range(3):
            for kv in range(3):
                for kw in range(3):
                    g = _g27(kt, kv, kw)
                    W1G[:64, pi, g, :] = g1[2 * pi, :, :, kt, kv, kw].T
                    W1G[64:, pi, g, :] = g1[2 * pi + 1, :, :, kt, kv, kw].T
    # conv2 weights: F(2,3) transform along ku, per (mode, kt, kv, kw)
    g2 = np.einsum('mu,oituvw->moitvw', G_W, w2)
    W2G = np.zeros((128, 4, 27, 64), np.float16)
    for m in range(4):
        for kt in range(3):
            for kv in range(3):
                for kw in range(3):
                    g = _g27(kt, kv, kw)
                    W2G[:, m, g, :] = g2[m, :, :, kt, kv, kw].T
    return dict(X=Xs, MT=MTs, MB=MBs,
                W1G=W1G.reshape(128, 2 * 27 * 128),
                W2G=W2G.reshape(128, 4 * 27 * 64),
                B1=np.asarray(b1, np.float32).reshape(128, 1),
                B2=np.asarray(b2, np.float32).reshape(64, 1))


def _build_module():
    import concourse.bass as bass
    import concourse.tile as tile
    from concourse import bacc, mybir

    fp16 = mybir.dt.float16
    fp32 = mybir.dt.float32

    nc = bacc.Bacc("TRN2", target_bir_lowering=False, debug=False, num_devices=1)
    xA_d = nc.dram_tensor("xA", [64, T, 2, 8 * XDP], fp16, kind="ExternalInput")
    xB_d = nc.dram_tensor("xB", [64, T, 2, 8 * XDP], fp16, kind="ExternalInput")
    w1g_d = nc.dram_tensor("w1g", [128, 2 * 27 * 128], fp16, kind="ExternalInput")
    w2g_d = nc.dram_tensor("w2g", [128, 4 * 27 * 64], fp16, kind="ExternalInput")
    b1_d = nc.dram_tensor("b1", [128, 1], fp32, kind="ExternalInput")
    b2_d = nc.dram_tensor("b2", [64, 1], fp32, kind="ExternalInput")
    mt_d = nc.dram_tensor("mt", [128, 1], fp32, kind="ExternalInput")
    mb_d = nc.dram_tensor("mb", [128, 1], fp32, kind="ExternalInput")
    y_d = nc.dram_tensor("y", [64, T, D * SH * W], fp32, kind="ExternalOutput")

    with tile.TileContext(nc) as tc:
        with (
            tc.tile_pool(name="xw", bufs=1) as xw,
            tc.tile_pool(name="hp", bufs=1) as hpool,
            tc.tile_pool(name="st", bufs=4) as stp,
            tc.tile_pool(name="p1", bufs=2, space="PSUM") as p1,
            tc.tile_pool(name="p2", bufs=2, space="PSUM") as p2,
        ):
            w1g = xw.tile([128, 2, 27, 128], fp16)
            nc.sync.dma_start(w1g[:, 0, :, :], w1g_d.ap()[:, 0:27 * 128])
            b1 = xw.tile([128, 1], fp32)
            nc.sync.dma_start(b1[:, :], b1_d.ap())

            # x-mode planes: top partition half holds modes (0,2) for chain
            # A, bottom half modes (1,3) for chain B. The h-mode planes for
            # conv2's pair-1 reuse this region (tag alias) once conv1 is done.
            # DMA in consumption order: pair-0 planes of t=0,1 gate the start.
            xq = xw.tile([128, T, 2, 8, XH, XROW], fp16, tag="xh")

            def xchunk(t, mi):
                nc.sync.dma_start(xq[0:64, t, mi, :, :, :],
                                  xA_d.ap()[:, t, mi, :])
                nc.sync.dma_start(xq[64:128, t, mi, :, :, :],
                                  xB_d.ap()[:, t, mi, :])

            xchunk(0, 0)
            xchunk(1, 0)
            nc.sync.dma_start(w1g[:, 1, :, :],
                              w1g_d.ap()[:, 27 * 128:2 * 27 * 128])
            xchunk(0, 1)
            xchunk(1, 1)
            mt = xw.tile([128, 1], fp32)
            nc.sync.dma_start(mt[:, :], mt_d.ap())
            mb = xw.tile([128, 1], fp32)
            nc.sync.dma_start(mb[:, :], mb_d.ap())
            for t in (2, 3):
                xchunk(t, 0)
                xchunk(t, 1)

            w2g = xw.tile([128, 4, 27, 64], fp16)
            nc.sync.dma_start(w2g[:, :, :, :], w2g_d.ap())
            b2 = xw.tile([64, 1], fp32)
            nc.sync.dma_start(b2[:, :], b2_d.ap())

            ht = hpool.tile([128, T, D, HHH, HW_], fp16)
            for t in range(T):
                nc.vector.memset(ht[:, t, :, :, 0:1], 0.0)
                nc.vector.memset(ht[:, t, :, :, 33:34], 0.0)
            # h-mode planes for conv2 pair 0 (modes 0,1); pair 1 (modes 2,3)
            # lives in hb1, aliased over xq
            hb0 = hpool.tile([128, T, 2, 8, HHH, HW_], fp16)

            # ---- conv1 (D-Winograd) ----
            # per (t, dtile): mode pair (0,1) accumulates on chains A/B,
            # modes staged to SBUF (bias folded into mode 1), then pair
            # (2,3); inverse: h[2j] = M0+M1+M2, h[2j+1] = M1-M2-M3
            for t in range(T):
                taps = [(kt, kv, kw) for kt in _t_taps(t)
                        for kv in range(3) for kw in range(3)]
                n = len(taps)
                for j in range(8):
                    ms0 = stp.tile([128, HHH, 32], fp32)
                    ms1 = stp.tile([128, HHH, 32], fp32)
                    for pair in range(2):
                        psA = p1.tile([128, HHH, W], fp32)
                        psB = p1.tile([128, HHH, W], fp32)
                        for i, (kt, kv, kw) in enumerate(taps):
                            tp = t + kt - 1
                            g = _g27(kt, kv, kw)
                            nc.tensor.matmul(
                                psA[:, :, :], w1g[0:64, pair, g, :],
                                xq[0:64, tp, pair, j, kv:kv + HHH, kw:kw + W],
                                start=(i == 0), stop=(i == n - 1),
                                tile_position=(0, 0))
                            nc.tensor.matmul(
                                psB[:, :, :], w1g[64:128, pair, g, :],
                                xq[64:128, tp, pair, j, kv:kv + HHH, kw:kw + W],
                                start=(i == 0), stop=(i == n - 1),
                                tile_position=(64, 0))
                        if pair == 0:
                            # stage modes 0,1; fold bias into mode 1 so both
                            # inverse outputs carry it exactly once
                            nc.vector.tensor_scalar_add(
                                ms0[:, :, :], psA[:, :, :], 0.0)
                            nc.scalar.activation(
                                ms1[:, :, :], psB[:, :, :],
                                mybir.ActivationFunctionType.Identity,
                                bias=b1[:, 0:1])
                        else:
                            nc.vector.tensor_add(ms0[:, :, :], ms0[:, :, :],
                                                 ms1[:, :, :])
                            nc.vector.tensor_add(ms0[:, :, :], ms0[:, :, :],
                                                 psA[:, :, :])
                            nc.scalar.activation(
                                ht[:, t, 2 * j, :, 1:33], ms0[:, :, :],
                                mybir.ActivationFunctionType.Relu)
                            nc.vector.tensor_sub(ms1[:, :, :], ms1[:, :, :],
                                                 psA[:, :, :])
                            nc.vector.tensor_sub(ms1[:, :, :], ms1[:, :, :],
                                                 psB[:, :, :])
                            nc.scalar.activation(
                                ht[:, t, 2 * j + 1, :, 1:33], ms1[:, :, :],
                                mybir.ActivationFunctionType.Relu)
                # zero out-of-image h halo rows (mask is 0 only on edge cores)
                nc.vector.tensor_scalar_mul(
                    ht[:, t, :, 0, 1:33], ht[:, t, :, 0, 1:33], mt[:, 0:1])
                nc.vector.tensor_scalar_mul(
                    ht[:, t, :, HHH - 1, 1:33], ht[:, t, :, HHH - 1, 1:33],
                    mb[:, 0:1])
                # conv2 forward F(2,3) transform along d, pair-0 modes:
                # m0[jd] = h[2jd-1]-h[2jd+1] (h[-1]=0), m1[jd] = h[2jd]+h[2jd+1]
                nc.vector.tensor_scalar_mul(
                    hb0[:, t, 0, 0, :, :], ht[:, t, 1, :, :], -1.0)
                nc.vector.tensor_sub(
                    hb0[:, t, 0, 1:8, :, :], ht[:, t, 1:14:2, :, :],
                    ht[:, t, 3:16:2, :, :])
                nc.vector.tensor_add(
                    hb0[:, t, 1, :, :, :], ht[:, t, 0:15:2, :, :],
                    ht[:, t, 1:16:2, :, :])

            # pair-1 modes (reuse the x-mode region, conv1 is done with it):
            # m2[jd] = h[2jd+1]-h[2jd], m3[jd] = h[2jd]-h[2jd+2] (h[16]=0)
            hb1 = xw.tile([128, T, 2, 8, HHH, HW_], fp16, tag="xh")
            for t in range(T):
                nc.vector.tensor_sub(
                    hb1[:, t, 0, :, :, :], ht[:, t, 1:16:2, :, :],
                    ht[:, t, 0:15:2, :, :])
                nc.vector.tensor_sub(
                    hb1[:, t, 1, 0:7, :, :], ht[:, t, 0:13:2, :, :],
                    ht[:, t, 2:15:2, :, :])
                nc.vector.tensor_copy(
                    hb1[:, t, 1, 7, :, :], ht[:, t, 14, :, :])

            # ---- conv2 (D-Winograd) ----
            # per (t, jd-pair): mode pair (0,1) then (2,3); within a pair the
            # two modes run on PE column groups (0,0)/(0,64) with K=128,
            # N=512 (2 dtiles x 8 rows x 32). Inverse + bias on DVE/Scalar:
            # y[4jp+2js+0] = M0+M1+M2+b2, y[4jp+2js+1] = M1-M2-M3+b2
            for t in range(T):
                taps2 = [(kt, kv, kw) for kt in _t_taps(t)
                         for kv in range(3) for kw in range(3)]
                n2 = len(taps2)
                for jp in range(4):
                    ps1 = p2.tile([128, N2], fp32)
                    ps2t = p2.tile([128, N2], fp32)
                    for pst, hbp in ((ps1, hb0), (ps2t, hb1)):
                        mb_ = 0 if pst is ps1 else 2
                        for i, (kt, kv, kw) in enumerate(taps2):
                            tp = t + kt - 1
                            g = _g27(kt, kv, kw)
                            for mi, base, tpos in ((0, 0, (0, 0)),
                                                   (1, 64, (0, 64))):
                                rhs = hbp[:, tp, mi, 2 * jp:2 * jp + 2,
                                          kv:kv + SH, kw:kw + W]
                                nc.tensor.matmul(
                                    pst[base:base + 64, 0:N2],
                                    w2g[:, mb_ + mi, g, :], rhs,
                                    start=(i == 0), stop=(i == n2 - 1),
                                    tile_position=tpos)
                    s0 = stp.tile([64, N2], fp32, bufs=2)
                    s1 = stp.tile([64, N2], fp32, bufs=2)
                    nc.vector.tensor_scalar_add(s0[:, :], ps1[0:64, :], 0.0)
                    nc.scalar.activation(
                        s1[:, :], ps1[64:128, :],
                        mybir.ActivationFunctionType.Identity, bias=b2[:, 0:1])
                    nc.vector.tensor_add(s0[:, :], s0[:, :], s1[:, :])
                    nc.vector.tensor_add(s0[:, :], s0[:, :], ps2t[0:64, :])
                    nc.vector.tensor_sub(s1[:, :], s1[:, :], ps2t[0:64, :])
                    nc.vector.tensor_sub(s1[:, :], s1[:, :], ps2t[64:128, :])
                    qy = 4 * jp * 256
                    nc.sync.dma_start(y_d.ap()[:, t, qy:qy + 256],
                                      s0[:, 0:256])
                    nc.sync.dma_start(y_d.ap()[:, t, qy + 512:qy + 768],
                                      s0[:, 256:512])
                    nc.sync.dma_start(y_d.ap()[:, t, qy + 256:qy + 512],
                                      s1[:, 0:256])
                    nc.sync.dma_start(y_d.ap()[:, t, qy + 768:qy + 1024],
                                      s1[:, 256:512])
    nc.compile()
    return nc


def _in_map(hostd, core):
    return {
        "xA": hostd["X"][core][0], "xB": hostd["X"][core][1],
        "mt": hostd["MT"][core], "mb": hostd["MB"][core],
        "w1g": hostd["W1G"], "w2g": hostd["W2G"],
        "b1": hostd["B1"], "b2": hostd["B2"],
    }


def kernel(x, w1, b1, w2, b2):
    from concourse.bass_utils import run_bass_kernel_spmd

    hostd = _make_host_arrays(x, w1, b1, w2, b2)
    if "nc" not in _cache:
        _cache["nc"] = _build_module()
    nc = _cache["nc"]

    in_maps = [_in_map(hostd, core) for core in range(NCORES)]
    res = run_bass_kernel_spmd(nc, in_maps, core_ids=list(range(NCORES)))

    y = np.zeros((B, C_OUT, T, D, H, W), np.float32)
    for core in range(NCORES):
        b, j = divmod(core, NJ)
        yc = res.results[core]["y"].reshape(C_OUT, T, D, SH, W)
        y[b, :, :, :, SH * j:SH * (j + 1), :] = yc
    return y


# revision 29
# speedup vs baseline: 1.4613x; 1.0242x over previous
"""4D Conv-MLP (conv3^4 -> ReLU -> conv3^4) on 8 Trainium2 NeuronCores.

Sharding: core = b*4 + j (batch b in {0,1}, H-slab j in {0..3}, 8 output rows
each). conv1 is recomputed on a 1-row h halo, so no cross-core communication.

F(2,3) Winograd along D for conv1 (input transform done on host, for free):
the ku tap dimension is replaced by 4 Winograd modes over 8 d-tiles of 2.
Each tap (kt,kv,kw) of each mode is a K=64, M=128 matmul; modes pair up on
the PE's two row-tile chains (tile_position (0,0)/(64,0)) which stream
concurrently, so C_in=64 runs at full array rate with no packed copies.
Mode pairs (0,1) then (2,3) accumulate per (t, dtile); modes 0,1 are staged
to SBUF (bias folded into mode 1), then the F(2,3) inverse + ReLU produce
the two h d-planes on DVE+Scalar while the PE streams the next tile.

conv2 stays direct: N=512 d-pair runs over 81 taps, K=128, M=64 (full rate);
taps alternate PE column groups via tile_position (0,0)/(0,64).
"""

import numpy as np

B, C_IN, C_HID, C_OUT = 2, 64, 128, 64
T, D, H, W = 4, 16, 32, 32
NCORES, NJ = 8, 4
SH = H // NJ          # 8 out rows per slab
XH = SH + 4           # 12 x rows per slab
HHH = SH + 2          # 10 h rows per slab
XROW = 34             # padded W
XDP = XH * XROW       # 408 = one (m, dtile) plane
MP = 1 + 2 * 8 * XDP + 7   # x-mode tile cols per half (2 modes)
HW_ = 34
N1 = HHH * XROW       # conv1 run = 340
N2 = 512              # conv2 run (2 dtiles x 8 x 32)

BT_W = np.array([[1, 0, -1, 0], [0, 1, 1, 0], [0, -1, 1, 0], [0, 1, 0, -1]],
                np.float32)
G_W = np.array([[1, 0, 0], [.5, .5, .5], [.5, -.5, .5], [0, 0, 1]], np.float32)

_cache = {}


def _t_taps(t):
    return [kt for kt in range(3) if 0 <= t + kt - 1 < T]


def _g27(kt, kv, kw):
    return (kt * 3 + kv) * 3 + kw


def _g81(kt, ku, kv, kw):
    return ((kt * 3 + ku) * 3 + kv) * 3 + kw


def _make_host_arrays(x, w1, b1, w2, b2):
    x = np.asarray(x, np.float32)
    Xs, MTs, MBs = [], [], []
    for core in range(NCORES):
        b, j = divmod(core, NJ)
        h0 = SH * j
        # x slab with h halo, then zero-pad d (+-1) and w (+-1)
        xpad = np.zeros((C_IN, T, D + 2, XH, XROW), np.float32)
        lo, hi = h0 - 2, h0 + 10
        slo, shi = max(lo, 0), min(hi, H)
        xpad[:, :, 1:17, slo - lo:shi - lo, 1:33] = x[b, :, :, :, slo:shi, :]
        # host F(2,3) forward transform along D: dtile jd covers out d
        # (2jd, 2jd+1), reads xpad d-rows 2jd..2jd+3
        xt = np.empty((C_IN, T, 4, 8, XH, XROW), np.float32)
        for jd in range(8):
            P = xpad[:, :, 2 * jd:2 * jd + 4]
            for m in range(4):
                xt[:, :, m, jd] = np.einsum('a,ctahw->cthw', BT_W[m], P)
        # chain A (PE rows 0:64) consumes modes 0,2; chain B modes 1,3
        XA = xt[:, :, 0::2].reshape(C_IN, T, -1).astype(np.float16)
        XB = xt[:, :, 1::2].reshape(C_IN, T, -1).astype(np.float16)
        Xs.append((XA, XB))
        MTs.append(np.full((128, 1), 0.0 if j == 0 else 1.0, np.float32))
        MBs.append(np.full((128, 1), 0.0 if j == NJ - 1 else 1.0, np.float32))

    w1 = np.asarray(w1, np.float32)
    w2 = np.asarray(w2, np.float32)
    # conv1 weights: F(2,3) transform along ku; top partition half serves
    # chain A (modes 0,2), bottom half chain B (modes 1,3)
    g1 = np.einsum('mu,oituvw->moitvw', G_W, w1)
    W1G = np.zeros((128, 2, 27, 128), np.float16)
    for pi in range(2):
        for kt in range(3):
            for kv in range(3):
                for kw in range(3):
                    g = _g27(kt, kv, kw)
                    W1G[:64, pi, g, :] = g1[2 * pi, :, :, kt, kv, kw].T
                    W1G[64:, pi, g, :] = g1[2 * pi + 1, :, :, kt, kv, kw].T
    # conv2 weights: F(2,3) transform along ku, per (mode, kt, kv, kw)
    g2 = np.einsum('mu,oituvw->moitvw', G_W, w2)
    W2G = np.zeros((128, 4, 27, 64), np.float16)
    for m in range(4):
        for kt in range(3):
            for kv in range(3):
                for kw in range(3):
                    g = _g27(kt, kv, kw)
                    W2G[:, m, g, :] = g2[m, :, :, kt, kv, kw].T
    return dict(X=Xs, MT=MTs, MB=MBs,
                W1G=W1G.reshape(128, 2 * 27 * 128),
                W2G=W2G.reshape(128, 4 * 27 * 64),
                B1=np.asarray(b1, np.float32).reshape(128, 1),
                B2=np.asarray(b2, np.float32).reshape(64, 1))


def _build_module():
    import concourse.bass as bass
    import concourse.tile as tile
    from concourse import bacc, mybir

    fp16 = mybir.dt.float16
    fp32 = mybir.dt.float32

    nc = bacc.Bacc("TRN2", target_bir_lowering=False, debug=False, num_devices=1)
    xA_d = nc.dram_tensor("xA", [64, T, 2, 8 * XDP], fp16, kind="ExternalInput")
    xB_d = nc.dram_tensor("xB", [64, T, 2, 8 * XDP], fp16, kind="ExternalInput")
    w1g_d = nc.dram_tensor("w1g", [128, 2 * 27 * 128], fp16, kind="ExternalInput")
    w2g_d = nc.dram_tensor("w2g", [128, 4 * 27 * 64], fp16, kind="ExternalInput")
    b1_d = nc.dram_tensor("b1", [128, 1], fp32, kind="ExternalInput")
    b2_d = nc.dram_tensor("b2", [64, 1], fp32, kind="ExternalInput")
    mt_d = nc.dram_tensor("mt", [128, 1], fp32, kind="ExternalInput")
    mb_d = nc.dram_tensor("mb", [128, 1], fp32, kind="ExternalInput")
    y_d = nc.dram_tensor("y", [64, T, D * SH * W], fp32, kind="ExternalOutput")

    with tile.TileContext(nc) as tc:
        with (
            tc.tile_pool(name="xw", bufs=1) as xw,
            tc.tile_pool(name="hp", bufs=1) as hpool,
            tc.tile_pool(name="st", bufs=4) as stp,
            tc.tile_pool(name="p1", bufs=2, space="PSUM") as p1,
            tc.tile_pool(name="p2", bufs=2, space="PSUM") as p2,
        ):
            # t=0 only uses taps kt in {1,2} (g >= 9), so that weight range
            # plus the first dtiles of (t,mi)=(0,0),(1,0) gate the start
            w1g = xw.tile([128, 2, 27, 128], fp16)

            def wchunk(pi, g0, g1):
                nc.sync.dma_start(w1g[:, pi, g0:g1, :],
                                  w1g_d.ap()[:, (27 * pi + g0) * 128:
                                             (27 * pi + g1) * 128])

            wchunk(0, 9, 27)
            b1 = xw.tile([128, 1], fp32)
            nc.sync.dma_start(b1[:, :], b1_d.ap())

            # x-mode planes: top partition half holds modes (0,2) for chain
            # A, bottom half modes (1,3) for chain B. The h-mode planes for
            # conv2's pair-1 reuse this region (tag alias) once conv1 is done.
            xq = xw.tile([128, T, 2, 8, XH, XROW], fp16, tag="xh")

            def xchunk(t, mi, j0, j1):
                nc.sync.dma_start(xq[0:64, t, mi, j0:j1, :, :],
                                  xA_d.ap()[:, t, mi, j0 * XDP:j1 * XDP])
                nc.sync.dma_start(xq[64:128, t, mi, j0:j1, :, :],
                                  xB_d.ap()[:, t, mi, j0 * XDP:j1 * XDP])

            xchunk(0, 0, 0, 2)
            xchunk(1, 0, 0, 2)
            wchunk(1, 9, 27)
            xchunk(0, 1, 0, 2)
            xchunk(1, 1, 0, 2)
            for mi in (0, 1):
                xchunk(0, mi, 2, 8)
                xchunk(1, mi, 2, 8)
            wchunk(0, 0, 9)
            wchunk(1, 0, 9)
            mt = xw.tile([128, 1], fp32)
            nc.sync.dma_start(mt[:, :], mt_d.ap())
            mb = xw.tile([128, 1], fp32)
            nc.sync.dma_start(mb[:, :], mb_d.ap())
            for t in (2, 3):
                xchunk(t, 0, 0, 8)
                xchunk(t, 1, 0, 8)

            w2g = xw.tile([128, 4, 27, 64], fp16)
            nc.sync.dma_start(w2g[:, :, :, :], w2g_d.ap())
            b2 = xw.tile([64, 1], fp32)
            nc.sync.dma_start(b2[:, :], b2_d.ap())

            ht = hpool.tile([128, T, D, HHH, HW_], fp16)
            for t in range(T):
                nc.vector.memset(ht[:, t, :, :, 0:1], 0.0)
                nc.vector.memset(ht[:, t, :, :, 33:34], 0.0)
            # h-mode planes for conv2 pair 0 (modes 0,1); pair 1 (modes 2,3)
            # lives in hb1, aliased over xq
            hb0 = hpool.tile([128, T, 2, 8, HHH, HW_], fp16)

            # ---- conv1 (D-Winograd) ----
            # per (t, dtile): mode pair (0,1) accumulates on chains A/B,
            # modes staged to SBUF (bias folded into mode 1), then pair
            # (2,3); inverse: h[2j] = M0+M1+M2, h[2j+1] = M1-M2-M3
            for t in range(T):
                taps = [(kt, kv, kw) for kt in _t_taps(t)
                        for kv in range(3) for kw in range(3)]
                n = len(taps)
                for j in range(8):
                    ms0 = stp.tile([128, HHH, 32], fp32)
                    ms1 = stp.tile([128, HHH, 32], fp32)
                    for pair in range(2):
                        psA = p1.tile([128, HHH, W], fp32)
                        psB = p1.tile([128, HHH, W], fp32)
                        for i, (kt, kv, kw) in enumerate(taps):
                            tp = t + kt - 1
                            g = _g27(kt, kv, kw)
                            nc.tensor.matmul(
                                psA[:, :, :], w1g[0:64, pair, g, :],
                                xq[0:64, tp, pair, j, kv:kv + HHH, kw:kw + W],
                                start=(i == 0), stop=(i == n - 1),
                                tile_position=(0, 0))
                            nc.tensor.matmul(
                                psB[:, :, :], w1g[64:128, pair, g, :],
                                xq[64:128, tp, pair, j, kv:kv + HHH, kw:kw + W],
                                start=(i == 0), stop=(i == n - 1),
                                tile_position=(64, 0))
                        if pair == 0:
                            # stage modes 0,1; fold bias into mode 1 so both
                            # inverse outputs carry it exactly once
                            nc.vector.tensor_scalar_add(
                                ms0[:, :, :], psA[:, :, :], 0.0)
                            nc.scalar.activation(
                                ms1[:, :, :], psB[:, :, :],
                                mybir.ActivationFunctionType.Identity,
                                bias=b1[:, 0:1])
                        else:
                            nc.vector.tensor_add(ms0[:, :, :], ms0[:, :, :],
                                                 ms1[:, :, :])
                            nc.vector.tensor_add(ms0[:, :, :], ms0[:, :, :],
                                                 psA[:, :, :])
                            nc.scalar.activation(
                                ht[:, t, 2 * j, :, 1:33], ms0[:, :, :],
                                mybir.ActivationFunctionType.Relu)
                            nc.vector.tensor_sub(ms1[:, :, :], ms1[:, :, :],
                                                 psA[:, :, :])
                            nc.vector.tensor_sub(ms1[:, :, :], ms1[:, :, :],
                                                 psB[:, :, :])
                            nc.scalar.activation(
                                ht[:, t, 2 * j + 1, :, 1:33], ms1[:, :, :],
                                mybir.ActivationFunctionType.Relu)
                # zero out-of-image h halo rows (mask is 0 only on edge cores)
                nc.vector.tensor_scalar_mul(
                    ht[:, t, :, 0, 1:33], ht[:, t, :, 0, 1:33], mt[:, 0:1])
                nc.vector.tensor_scalar_mul(
                    ht[:, t, :, HHH - 1, 1:33], ht[:, t, :, HHH - 1, 1:33],
                    mb[:, 0:1])
                # conv2 forward F(2,3) transform along d, pair-0 modes:
                # m0[jd] = h[2jd-1]-h[2jd+1] (h[-1]=0), m1[jd] = h[2jd]+h[2jd+1]
                nc.vector.tensor_scalar_mul(
                    hb0[:, t, 0, 0, :, :], ht[:, t, 1, :, :], -1.0)
                nc.vector.tensor_sub(
                    hb0[:, t, 0, 1:8, :, :], ht[:, t, 1:14:2, :, :],
                    ht[:, t, 3:16:2, :, :])
                nc.vector.tensor_add(
                    hb0[:, t, 1, :, :, :], ht[:, t, 0:15:2, :, :],
                    ht[:, t, 1:16:2, :, :])

            # pair-1 modes (reuse the x-mode region, conv1 is done with it):
            # m2[jd] = h[2jd+1]-h[2jd], m3[jd] = h[2jd]-h[2jd+2] (h[16]=0)
            hb1 = xw.tile([128, T, 2, 8, HHH, HW_], fp16, tag="xh")
            for t in range(T):
                nc.vector.tensor_sub(
                    hb1[:, t, 0, :, :, :], ht[:, t, 1:16:2, :, :],
                    ht[:, t, 0:15:2, :, :])
                nc.vector.tensor_sub(
                    hb1[:, t, 1, 0:7, :, :], ht[:, t, 0:13:2, :, :],
                    ht[:, t, 2:15:2, :, :])
                nc.vector.tensor_copy(
                    hb1[:, t, 1, 7, :, :], ht[:, t, 14, :, :])

            # ---- conv2 (D-Winograd) ----
            # per (t, jd-pair): mode pair (0,1) then (2,3); within a pair the
            # two modes run on PE column groups (0,0)/(0,64) with K=128,
            # N=512 (2 dtiles x 8 rows x 32). Inverse + bias on DVE/Scalar:
            # y[4jp+2js+0] = M0+M1+M2+b2, y[4jp+2js+1] = M1-M2-M3+b2
            for t in range(T):
                taps2 = [(kt, kv, kw) for kt in _t_taps(t)
                         for kv in range(3) for kw in range(3)]
                n2 = len(taps2)
                for jp in range(4):
                    ps1 = p2.tile([128, N2], fp32)
                    ps2t = p2.tile([128, N2], fp32)
                    for pst, hbp in ((ps1, hb0), (ps2t, hb1)):
                        mb_ = 0 if pst is ps1 else 2
                        for i, (kt, kv, kw) in enumerate(taps2):
                            tp = t + kt - 1
                            g = _g27(kt, kv, kw)
                            for mi, base, tpos in ((0, 0, (0, 0)),
                                                   (1, 64, (0, 64))):
                                rhs = hbp[:, tp, mi, 2 * jp:2 * jp + 2,
                                          kv:kv + SH, kw:kw + W]
                                nc.tensor.matmul(
                                    pst[base:base + 64, 0:N2],
                                    w2g[:, mb_ + mi, g, :], rhs,
                                    start=(i == 0), stop=(i == n2 - 1),
                                    tile_position=tpos)
                    s0 = stp.tile([64, N2], fp32, bufs=2)
                    s1 = stp.tile([64, N2], fp32, bufs=2)
                    nc.vector.tensor_scalar_add(s0[:, :], ps1[0:64, :], 0.0)
                    nc.scalar.activation(
                        s1[:, :], ps1[64:128, :],
                        mybir.ActivationFunctionType.Identity, bias=b2[:, 0:1])
                    nc.vector.tensor_add(s0[:, :], s0[:, :], s1[:, :])
                    nc.vector.tensor_add(s0[:, :], s0[:, :], ps2t[0:64, :])
                    nc.vector.tensor_sub(s1[:, :], s1[:, :], ps2t[0:64, :])
                    nc.vector.tensor_sub(s1[:, :], s1[:, :], ps2t[64:128, :])
                    qy = 4 * jp * 256
                    nc.sync.dma_start(y_d.ap()[:, t, qy:qy + 256],
                                      s0[:, 0:256])
                    nc.sync.dma_start(y_d.ap()[:, t, qy + 512:qy + 768],
                                      s0[:, 256:512])
                    nc.sync.dma_start(y_d.ap()[:, t, qy + 256:qy + 512],
                                      s1[:, 0:256])
                    nc.sync.dma_start(y_d.ap()[:, t, qy + 768:qy + 1024],
                                      s1[:, 256:512])
    nc.compile()
    return nc


def _in_map(hostd, core):
    return {
        "xA": hostd["X"][core][0], "xB": hostd["X"][core][1],
        "mt": hostd["MT"][core], "mb": hostd["MB"][core],
        "w1g": hostd["W1G"], "w2g": hostd["W2G"],
        "b1": hostd["B1"], "b2": hostd["B2"],
    }


def kernel(x, w1, b1, w2, b2):
    from concourse.bass_utils import run_bass_kernel_spmd

    hostd = _make_host_arrays(x, w1, b1, w2, b2)
    if "nc" not in _cache:
        _cache["nc"] = _build_module()
    nc = _cache["nc"]

    in_maps = [_in_map(hostd, core) for core in range(NCORES)]
    res = run_bass_kernel_spmd(nc, in_maps, core_ids=list(range(NCORES)))

    y = np.zeros((B, C_OUT, T, D, H, W), np.float32)
    for core in range(NCORES):
        b, j = divmod(core, NJ)
        yc = res.results[core]["y"].reshape(C_OUT, T, D, SH, W)
        y[b, :, :, :, SH * j:SH * (j + 1), :] = yc
    return y
